# revision 12
# baseline (speedup 1.0000x reference)
"""VQ codebook encoding kernel for Trainium2, sharded over 8 NeuronCores.

Math (per shard of N tokens):
    l2[n,k]  = ||x_n - c_k||            (NOT squared)
    W        = softmax_k(l2 * scale_k)
    E[k,d]   = sum_n W[n,k] * (x[n,d] - c[k,d])
             = (W^T X)[k,d] - S_k * c[k,d],   S_k = sum_n W[n,k]

Key algebra: logits = s_k * l2 = sqrt(s_k^2 * l2^2), and
    s^2 l2^2 = (x . ct2s) + s2_k*||x_n||^2 + s2_k*||c_k||^2
The whole quadratic accumulates in PSUM on the PE:
  - per 128-token tile: scores += x_tile^T(f8) . ct2s(f8)
  - per chunk, ONE rank-(n+1) matmul adds the s2*xx + s2*cc terms:
      lhs[r,p] = xx[tile r, token p] (r<n), ones (r=n)     [fp16]
      rhs[r, i*K+k] = s2[k]*[r==i]   (r<n), s2[k]cc[k] (r=n)
so A = s^2 l2^2 sits in PSUM f32 with no DVE add and no 131KB ccxx DMA.
On-chip: EW = exp(exp(0.5 ln A)) (ONE activation table -- the compile
patches bass's act-table map so Ln/Exp resolve only to
"natural_log_exp_and_others"; the stock placement loads exp_and_others
too, costing ~1.3us of DMA-engine time during the input-critical phase).
W = EW * (1/den).  The device returns [M | S] = W^T [X|1] per chunk
(separate PSUM banks + separate output DMAs); the host finishes
E = sum M - (sum S) * codes.

Performance structure:
- ACT ladder (6 transcendental passes) is the saturated mid-phase
  resource; LN reads A straight from PSUM.
- input DMAs: xa (ct2s + x^T, f8) on sync queue; xk (xx/rank-terms,
  fp16, 9KB) then xa chunk1 on scalar queue; xn on gpsimd queue.
- all matmul operands 8/16-bit (1 cycle/row on the PE vs 4 for fp32).
- fp16 xx/s2cc: A needs ~0.1 abs accuracy at ~300 -> fp16 ok; A > 0
  always (128-dim gaussians never collide), so ln never sees <= 0.
"""

import sys

if "/opt/trn_rl_repo" not in sys.path:
    sys.path.insert(0, "/opt/trn_rl_repo")

import numpy as np

N_CORES = 8
N, K, D = 16384, 32, 128
NPC = N // N_CORES          # tokens per core = 2048
NT = NPC // 128             # 128-token tiles per core = 16
XNW = 130                   # xn tile width: [x(128) | 1 | 0]

# chunk layout: (tile_offset, n_tiles).
CH = [(0, 8), (8, 8)]
NC_ = len(CH)
RMAX = max(n for _, n in CH) + 1   # rank rows per chunk (xx rows + ones)
# xk column layout: per-chunk lhs blocks (128 wide), then per-chunk rhs
# blocks (n*K wide)
XK_LHS = [c * 128 for c in range(NC_)]
XK_RHS = []
_off = NC_ * 128
for _, n in CH:
    XK_RHS.append(_off)
    _off += n * K
XKW = _off

_CACHE = {}


def _build_nc():
    import concourse.bacc as bacc
    import concourse.bass as bass
    import concourse.mybir as mybir

    f32 = mybir.dt.float32
    bf16 = mybir.dt.bfloat16
    f16 = mybir.dt.float16
    f8 = mybir.dt.float8e4
    AFT = mybir.ActivationFunctionType
    ALU = mybir.AluOpType

    nc = bacc.Bacc(None, target_bir_lowering=False)

    xa = nc.dram_tensor("xa", [128, K + NPC], f8, kind="ExternalInput")
    xk = nc.dram_tensor("xk", [RMAX, XKW], f16, kind="ExternalInput")
    xn = nc.dram_tensor("xn", [128, NT * XNW], f8, kind="ExternalInput")
    e_out = [
        nc.dram_tensor(f"E{c}", [K, XNW], f32, kind="ExternalOutput")
        for c in range(3)
    ]

    sb = lambda name, shape, dt: nc.alloc_sbuf_tensor(name, shape, dt)
    xa_sb = sb("xa_sb", [128, K + NPC], f8)
    xk_sb = sb("xk_sb", [RMAX, XKW], f16)
    xn_sb = sb("xn_sb", [128, NT * XNW], f8)
    l_t = [sb(f"l{c}", [128, n * K], f32) for c, (_, n) in enumerate(CH)]
    t_t = [sb(f"t{c}", [128, n * K], f32) for c, (_, n) in enumerate(CH)]
    ew_t = [sb(f"ew{c}", [128, n * K], bf16) for c, (_, n) in enumerate(CH)]
    den = [sb(f"den{c}", [128, n], f32) for c, (_, n) in enumerate(CH)]
    rden = [sb(f"rden{c}", [128, n], f32) for c, (_, n) in enumerate(CH)]
    w_t = [sb(f"w{c}", [128, n * K], f8) for c, (_, n) in enumerate(CH)]
    e_sb = [sb(f"e_sb{c}", [K, XNW], f32) for c in range(3)]

    # full-bank PSUM allocations (scores + aggregation per chunk)
    ps = [nc.alloc_psum_tensor(f"ps{c}", [128, 512], f32) for c in range(NC_)]
    pms = [nc.alloc_psum_tensor(f"pms{c}", [K, 512], f32) for c in range(3)]

    ct2v = xa_sb[:, 0:K]

    def bck(apw, count):
        # [128, w] per-tile scalars -> [128, w, count] via stride-0 inner dim
        return bass.AP(
            tensor=apw.tensor,
            offset=apw.offset,
            ap=[list(apw.ap[0]), list(apw.ap[1]), [0, count]],
        )

    def t3(ap, k=K):
        return ap.rearrange("p (t k) -> p t k", k=k)

    qA = [nc.alloc_semaphore(f"qA{c}") for c in range(NC_)]  # xa per chunk
    qX = nc.alloc_semaphore("qX")        # xk
    qN = nc.alloc_semaphore("qN")        # xn
    mmS = nc.alloc_semaphore("mmS")      # PE: score matmuls done, per chunk
    ewN = nc.alloc_semaphore("ewN")      # ACT: EW ready, per chunk
    wR = nc.alloc_semaphore("wR")        # DVE: W ready, per chunk
    aggS = nc.alloc_semaphore("aggS")    # PE: aggregation done, per chunk
    eR = [nc.alloc_semaphore(f"eR{c}") for c in range(3)]
    oD = nc.alloc_semaphore("oD")

    def xa_slice(off, n):
        lo = 0 if off == 0 else K + off * 128
        hi = K + (off + n) * 128
        return xa[:, lo:hi], xa_sb[:, lo:hi]

    with nc.Block(no_gpsimd_drain=True) as block:

        @block.sync
        def _(sync):
            # chunk0 (with ct2s) alone on this queue; chunk1 rides scalar
            src, dst = xa_slice(*CH[0])
            sync.dma_start(out=dst, in_=src).then_inc(qA[0], 16)
            for c in range(2):
                sync.wait_ge(eR[c], 1)
                # No completion wait: block-exit drain fences the queue.
                sync.dma_start(out=e_out[c][:, :], in_=e_sb[c][:, :]).then_inc(oD, 16)

        @block.scalar
        def _(scalar):
            scalar.dma_start(out=xk_sb[:, :], in_=xk[:, :]).then_inc(qX, 16)
            src, dst = xa_slice(*CH[1])
            scalar.dma_start(out=dst, in_=src).then_inc(qA[1], 16)
            for c, (_, n) in enumerate(CH):
                scalar.wait_ge(mmS, c + 1)
                nc.scalar.activation(
                    out=l_t[c][:, :], in_=ps[c][:, : n * K], func=AFT.Ln
                )
                scalar.drain()
                nc.scalar.activation(
                    out=t_t[c][:, :], in_=l_t[c][:, :], func=AFT.Exp, scale=0.5
                )
                scalar.drain()
                nc.scalar.activation(
                    out=ew_t[c][:, :], in_=t_t[c][:, :], func=AFT.Exp
                ).then_inc(ewN)
            # chunk0 epilogue on the (now idle) scalar engine
            scalar.wait_ge(aggS, 1)
            nc.scalar.activation(
                out=e_sb[0][:, :], in_=pms[0][:, :XNW], func=AFT.Copy
            ).then_inc(eR[0])
            scalar.wait_ge(eR[2], 1)
            scalar.dma_start(out=e_out[2][:, :], in_=e_sb[2][:, :]).then_inc(oD, 16)

        @block.gpsimd
        def _(gpsimd):
            # xn is the largest input but is not needed until aggregation
            # (~3us later); hold it back so xa0 has the engines to itself.
            gpsimd.wait_ge(qA[0], 16)
            gpsimd.dma_start(out=xn_sb[:, :], in_=xn[:, :]).then_inc(qN, 16)

        @block.tensor
        def _(tensor):
            for c, (off, n) in enumerate(CH):
                # rank-(n+1) base: s2_k xx_n + s2_k cc_k over the whole
                # chunk region (start=True), then the per-tile score
                # matmuls accumulate onto it (start=False) -- accumulation
                # only composes within one group, so the base goes first.
                if c == 0:
                    tensor.wait_ge(qX, 16)
                nc.tensor.matmul(
                    ps[c][:, : n * K],
                    xk_sb[0 : n + 1, XK_LHS[c] : XK_LHS[c] + 128],
                    xk_sb[0 : n + 1, XK_RHS[c] : XK_RHS[c] + n * K],
                    start=True, stop=False,
                )
                # tiles gate on xa only -- the rank base above already ran
                # while the x data was still in flight.
                tensor.wait_ge(qA[c], 16)
                for i in range(n):
                    mm = nc.tensor.matmul(
                        ps[c][:, i * K : (i + 1) * K],
                        xa_sb[:, K + (off + i) * 128 : K + (off + i + 1) * 128],
                        ct2v, start=False, stop=(i == n - 1),
                    )
                    if i == n - 1:
                        mm.then_inc(mmS)
            tensor.wait_ge(qN, 16)
            # chunk0: one W batch; chunk1: W arrives in halves (wR 2 then 3)
            # so aggregation starts before the second half-multiply finishes.
            off, n = CH[0]
            tensor.wait_ge(wR, 1)
            for i in range(n):
                mm = nc.tensor.matmul(
                    pms[0][:, :XNW],
                    w_t[0][:, i * K : (i + 1) * K],
                    xn_sb[:, (off + i) * XNW : (off + i + 1) * XNW],
                    start=(i == 0), stop=(i == n - 1),
                )
                if i == n - 1:
                    mm.then_inc(aggS)
            # chunk1: two independent half-aggregations into separate PSUM
            # regions; each half's copy+output DMA overlaps the other.
            off, n = CH[1]
            h = n // 2
            for half, lo in ((0, 0), (1, h)):
                tensor.wait_ge(wR, 2 + half)
                for j in range(h):
                    i = lo + j
                    mm = nc.tensor.matmul(
                        pms[1 + half][:, :XNW],
                        w_t[1][:, i * K : (i + 1) * K],
                        xn_sb[:, (off + i) * XNW : (off + i + 1) * XNW],
                        start=(j == 0), stop=(j == h - 1),
                    )
                    if j == h - 1:
                        mm.then_inc(aggS)

        @block.vector
        def _(vector):
            for c, (off, n) in enumerate(CH):
                vector.wait_ge(ewN, c + 1)
                # bf16 den: per-token bias ~0.4% averages out across tokens
                # in E (~0.01% net) -- far inside the error budget.
                with nc.allow_low_precision(reason="bf16 softmax denominator"):
                    nc.vector.tensor_reduce(
                        out=den[c][:, :], in_=t3(ew_t[c][:, :]),
                        axis=mybir.AxisListType.X, op=ALU.add,
                    )
                vector.drain()
                nc.vector.reciprocal_approx_fast(out=rden[c][:, :], in_=den[c][:, :])
                vector.drain()
                if c == 0:
                    nc.vector.tensor_mul(
                        t3(w_t[c][:, :]), t3(ew_t[c][:, :]), bck(rden[c][:, :], K)
                    ).then_inc(wR)
                else:
                    # halves, so the aggregation can start on the first half
                    h = n // 2
                    nc.vector.tensor_mul(
                        t3(w_t[c][:, : h * K]),
                        t3(ew_t[c][:, : h * K]),
                        bck(rden[c][:, :h], K),
                    ).then_inc(wR)
                    nc.vector.tensor_mul(
                        t3(w_t[c][:, h * K :]),
                        t3(ew_t[c][:, h * K :]),
                        bck(rden[c][:, h:], K),
                    ).then_inc(wR)
            for half in range(2):
                vector.wait_ge(aggS, 2 + half)
                nc.vector.tensor_copy(
                    e_sb[1 + half][:, :], pms[1 + half][:, :XNW]
                ).then_inc(eR[1 + half])

    _compile_single_act_table(nc, mybir)
    return nc


def _compile_single_act_table(nc, mybir):
    """Compile with Ln/Exp restricted to "natural_log_exp_and_others" so
    the act-table pass emits ONE InstLoadActFuncSet (stock placement also
    loads exp_and_others -- pure waste, and its TDRAM DMA occupies DMA
    engines during the input-critical phase).  get_activation_tables is
    re-read from act_info.json on every call, so mutating its return
    value does nothing; instead patch the name bacc's pass looks up,
    compile, and restore."""
    import concourse.bacc as bacc_mod

    AFT = mybir.ActivationFunctionType
    orig = bacc_mod.get_activation_tables

    def patched(arch):
        tables = orig(arch)
        for name, funcs in tables.items():
            if name != "natural_log_exp_and_others":
                funcs.discard(AFT.Exp)
                funcs.discard(AFT.Ln)
                funcs.discard(AFT.Copy)
        return tables

    bacc_mod.get_activation_tables = patched
    try:
        nc.compile()
    finally:
        bacc_mod.get_activation_tables = orig

    # The pass still emits an unconditional "load set 0" at block entry,
    # immediately superseded by the set-6 load before the first Ln (no
    # activation runs between them).  Drop it: its ~80KB TDRAM->table DMA
    # otherwise occupies DMA engines exactly when xa/xn are in flight.
    for func in nc.m.functions:
        for b in func.blocks:
            loads = [
                i for i in b.instructions
                if type(i).__name__ == "InstLoadActFuncSet"
            ]
            if len(loads) < 2:
                continue
            first_act = next(
                idx for idx, i in enumerate(b.instructions)
                if type(i).__name__ == "InstActivation"
            )
            for i in loads[:-1]:
                idx = b.instructions.index(i)
                assert idx < first_act and not (
                    i.sync_info and (i.sync_info.on_wait or i.sync_info.on_update)
                ), "unexpected sem links on dead act-table load"
                b.instructions.remove(i)


def _get_nc():
    if "nc" not in _CACHE:
        _CACHE["nc"] = _build_nc()
    return _CACHE["nc"]


def _prep_inputs(x, codes, scale):
    """Build the per-core input maps (all host-side numpy)."""
    import ml_dtypes

    f8 = ml_dtypes.float8_e4m3

    x = np.asarray(x, dtype=np.float32).reshape(N, D)
    codes = np.asarray(codes, dtype=np.float32)
    scale = np.asarray(scale, dtype=np.float32)

    s2 = (scale * scale).astype(np.float32)                         # [K]
    ct2s = np.ascontiguousarray(-2.0 * codes.T * s2[None, :])       # [D, K]
    ccs2 = ((codes * codes).sum(axis=1) * s2).astype(np.float32)    # [K]

    # shared rank-structure rhs blocks: [n+1, n*K] per chunk
    rk = np.zeros((RMAX, XKW - NC_ * 128), dtype=np.float32)
    for c, (_, n) in enumerate(CH):
        base = XK_RHS[c] - NC_ * 128
        for t in range(n):
            rk[t, base + t * K : base + (t + 1) * K] = s2
        rk[n, base : base + n * K] = np.tile(ccs2, n)

    in_maps = []
    for core in range(N_CORES):
        xs = x[core * NPC : (core + 1) * NPC]                       # [2048, 128]
        a = xs.reshape(128, NT, D)                                  # [p, t, d]
        xx = (a * a).sum(axis=2)                                    # [p, t]
        xtp = np.ascontiguousarray(a.transpose(2, 1, 0)).reshape(128, NPC)
        xav = np.concatenate([ct2s, xtp], axis=1)                   # [128, K+NPC]
        xkv = np.zeros((RMAX, XKW), dtype=np.float32)
        for c, (off, n) in enumerate(CH):
            xkv[0:n, XK_LHS[c] : XK_LHS[c] + 128] = xx[:, off : off + n].T
            xkv[n, XK_LHS[c] : XK_LHS[c] + 128] = 1.0
        xkv[:, NC_ * 128 :] = rk
        xnv = np.concatenate(
            [
                a,
                np.ones((128, NT, 1), dtype=np.float32),
                np.zeros((128, NT, 1), dtype=np.float32),
            ],
            axis=2,
        ).reshape(128, NT * XNW)
        in_maps.append(
            {
                "xa": np.ascontiguousarray(xav.astype(f8)),
                "xk": np.ascontiguousarray(xkv.astype(np.float16)),
                "xn": np.ascontiguousarray(xnv.astype(f8)),
            }
        )
    return in_maps


def _finish(results, codes):
    """Host-side epilogue: E = sum M - (sum S) * codes."""
    codes = np.asarray(codes, dtype=np.float32)
    acc = np.zeros((K, XNW), dtype=np.float64)
    for r in results:
        for c in range(3):
            acc += np.asarray(r[f"E{c}"], dtype=np.float64)
    out = acc[:, :D] - acc[:, D : D + 1] * codes.astype(np.float64)
    return out.astype(np.float32)


def kernel(x, codes, scale):
    from concourse.bass_utils import run_bass_kernel_spmd

    nc = _get_nc()
    in_maps = _prep_inputs(x, codes, scale)
    res = run_bass_kernel_spmd(nc, in_maps, core_ids=list(range(N_CORES)))
    return _finish(res.results, codes)
